# revision 7
# baseline (speedup 1.0000x reference)
"""Periodic-boundary fixed-capacity neighbour list on 8 trn2 NeuronCores.

Device algorithm (per core, 256 rows as 2 partition-tiles):
  For unit cell + cutoff 0.3, a pair (i, j) is within cutoff for at most ONE
  of the 27 periodic images, and per axis at most TWO image shifts are ever
  feasible for a given centre row ({0,+1} if p_i>0.5 else {-1,0}).  So the
  N x 27N reference mask collapses to N x N with a 3-bit reduced cell code:
     v_a = p_j,a - p_i,a          (ACT FMA; compare vs +-0.5 immediates)
     g_a = [v<-0.5] - [v>0.5]     (threshold-epsilon differences are no-hit
                                   safe: |w| would be ~0.5 >> 0.3)
     u_a = g_a + p_j,a ;  w_a = u_a - p_i,a    (fp32, reference-exact order)
     hit = ((wx^2+wy^2)+wz^2) <= 0.09f  and  j != i
     b_a = [u_a - flip2_a >= 0]   (exact: Sterbenz),  flip2_a = [p_i,a>0.5]
     key = (bz*4+by*2+bx)*2048 + j + 8192  in [8192, 24576) ; miss -> 32512
  Ascending-key order == the reference argwhere packing order.  Keys are
  cast to uint16 and REINTERPRETED as bf16 (positive-float bit patterns
  sort like integers) for a bitonic top-256 (sort eight 256-chunks, then
  reversed-read prune-merges 8->4->2->1) on the DVE.
  The j columns are stored BIT-ROTATED: physical column = q*8 + c8 for
  logical j = c8*256 + q (c8 = chunk id).  Phase 1 is column-permutation
  equivariant (host permutes p_j; the iota emits logical j values), and in
  phase 2 every chunk-local compare then has innermost AP [1,8] step-1 —
  unlocking the DVE 2x 16-bit mode for ALL stage-A substages including
  reversed-read merges.  Phase-1 is spread over ACT (FMA/Square, bitwise-
  IEEE verified), GPSIMD (tensor-tensor only) and DVE immediate compares.
Host: shard/replicate inputs, decode keys -> neighbours/cell_indices
(b-bit + flip rule -> shift vector), max of per-row hit counts ->
actual_max.  jnp.take(idx=-1) wraps: invalid cell slots = shifts[26] =
(1,1,1).
"""
import sys

if '/opt/trn_rl_repo' not in sys.path:
    sys.path.insert(0, '/opt/trn_rl_repo')

import numpy as np

N = 2048
K = 256
CHUNK = 256
NCH = 8  # chunks per row
NCORES = 8
ROWS_PER_CORE = N // NCORES  # 256
NTILES = ROWS_PER_CORE // 128  # 2
OFF = 8192
SENT = 32512  # 0x7F00 as uint16; huge finite positive as bf16
THR = 0.3 * 0.3  # fp32-converts to 0.090000004 like the jax reference

_cached = {}


def _build_program():
    import concourse.bacc as bacc
    import concourse.mybir as mybir
    from concourse.tile import TileContext

    f32 = mybir.dt.float32
    i32 = mybir.dt.int32
    u16 = mybir.dt.uint16
    bf16 = mybir.dt.bfloat16
    Alu = mybir.AluOpType
    Act = mybir.ActivationFunctionType
    Ax = mybir.AxisListType

    nc = bacc.Bacc("TRN2", target_bir_lowering=False)

    pjb_d = nc.dram_tensor("pjb", [3, N], f32, kind="ExternalInput")
    npi_d = nc.dram_tensor("npi", [NTILES, 128, 3], f32, kind="ExternalInput")
    nig_d = nc.dram_tensor("nig", [NTILES, 128, 1], f32, kind="ExternalInput")
    nfl_d = nc.dram_tensor("nfl", [NTILES, 128, 3], f32, kind="ExternalInput")
    keys_d = nc.dram_tensor("keys", [NTILES, 128, K], u16, kind="ExternalOutput")
    cnt_d = nc.dram_tensor("cnt", [NTILES, 128, 1], f32, kind="ExternalOutput")

    with TileContext(nc) as tc:
        with tc.tile_pool(name="main", bufs=1) as pool:
            big = [128, N]
            pj = [pool.tile(big, f32, name=f"pj{a}", tag=f"pj{a}") for a in range(3)]
            iota_i = pool.tile(big, i32, tag="iotai")
            iota_f = pool.tile(big, f32, tag="iotaf")

            for a in range(3):
                nc.sync.dma_start(
                    out=pj[a][:],
                    in_=pjb_d[a:a + 1, :].partition_broadcast(128).squeeze(1),
                )
            # value OFF + c8*256 + q written at physical column q*8 + c8
            nc.gpsimd.iota(iota_i[:], pattern=[[1, CHUNK], [CHUNK, NCH]],
                           base=OFF, channel_multiplier=0)
            nc.vector.tensor_copy(out=iota_f[:], in_=iota_i[:])

            for t in range(NTILES):
                npi_s = pool.tile([128, 3], f32, tag=f"npi{t}")
                nig_s = pool.tile([128, 1], f32, tag=f"nig{t}")
                nfl_s = pool.tile([128, 3], f32, tag=f"nfl{t}")
                cnt_s = pool.tile([128, 1], f32, tag=f"cnt{t}")
                nc.sync.dma_start(out=npi_s[:], in_=npi_d[t])
                nc.sync.dma_start(out=nig_s[:], in_=nig_d[t])
                nc.sync.dma_start(out=nfl_s[:], in_=nfl_d[t])

                u = [pool.tile(big, f32, name=f"u{a}", tag=f"u{a}") for a in range(3)]
                bt = [pool.tile(big, f32, name=f"bt{a}", tag=f"bt{a}") for a in range(3)]
                t1 = pool.tile(big, f32, tag="t1")
                t2 = pool.tile(big, f32, tag="t2")
                d2 = pool.tile(big, f32, tag="d2")
                hitm = pool.tile(big, f32, tag="hitm")
                noteq = pool.tile(big, f32, tag="noteq")

                for a in range(3):
                    va = pool.tile(big, f32, name=f"va{a}", tag="va")
                    cma = pool.tile(big, f32, name=f"cma{a}", tag="cma")
                    cmb = pool.tile(big, f32, name=f"cmb{a}", tag="cmb")
                    uba = pool.tile(big, f32, name=f"uba{a}", tag="uba")
                    # v = pj - pi  [ACT FMA, bit-exact]
                    nc.scalar.activation(va[:], pj[a][:], Act.Identity,
                                         bias=npi_s[:, a:a + 1], scale=1.0)
                    # g parts via immediate compares [DVE]
                    nc.vector.tensor_scalar(out=cma[:], in0=va[:],
                                            scalar1=-0.5, scalar2=None,
                                            op0=Alu.is_lt)
                    nc.vector.tensor_scalar(out=cmb[:], in0=va[:],
                                            scalar1=0.5, scalar2=None,
                                            op0=Alu.is_gt)
                    # g = cma - cmb ; u = g + pj   [GPSIMD]
                    nc.gpsimd.tensor_tensor(out=t1[:], in0=cma[:], in1=cmb[:],
                                            op=Alu.subtract)
                    nc.gpsimd.tensor_tensor(out=u[a][:], in0=t1[:],
                                            in1=pj[a][:], op=Alu.add)
                    # order bit: (u - flip2 >= 0) * 2^(11+a)
                    nc.scalar.activation(uba[:], u[a][:], Act.Identity,
                                         bias=nfl_s[:, a:a + 1], scale=1.0)
                    nc.vector.tensor_scalar(out=bt[a][:], in0=uba[:],
                                            scalar1=0.0,
                                            scalar2=float(2048 << a),
                                            op0=Alu.is_ge, op1=Alu.mult)
                    # w = u - pi ; sq = w^2  [ACT, bit-exact, in-place]
                    nc.scalar.activation(u[a][:], u[a][:], Act.Identity,
                                         bias=npi_s[:, a:a + 1], scale=1.0)
                    nc.scalar.activation(u[a][:], u[a][:], Act.Square)

                # d2 = (sq0 + sq1) + sq2   [DVE]
                nc.vector.tensor_tensor(out=t2[:], in0=u[0][:], in1=u[1][:],
                                        op=Alu.add)
                nc.vector.tensor_tensor(out=d2[:], in0=t2[:], in1=u[2][:],
                                        op=Alu.add)
                # hit = d2 <= THR [DVE imm]
                nc.vector.tensor_scalar(out=t2[:], in0=d2[:], scalar1=THR,
                                        scalar2=None, op0=Alu.is_le)
                # self-exclusion: iota != ig via ACT diff + imm compare
                vn = pool.tile(big, f32, name=f"vn{t}", tag="va")
                nc.scalar.activation(vn[:], iota_f[:], Act.Identity,
                                     bias=nig_s[:], scale=1.0)
                nc.vector.tensor_scalar(out=noteq[:], in0=vn[:], scalar1=0.0,
                                        scalar2=None, op0=Alu.not_equal)
                nc.gpsimd.tensor_tensor(out=hitm[:], in0=t2[:], in1=noteq[:],
                                        op=Alu.mult)
                nc.vector.tensor_reduce(out=cnt_s[:], in_=hitm[:], axis=Ax.X,
                                        op=Alu.add)
                nc.sync.dma_start(out=cnt_d[t], in_=cnt_s[:])

                # key' = bx' + by' + bz' + (8192 + j)    [GPSIMD adds]
                nc.gpsimd.tensor_tensor(out=t1[:], in0=bt[0][:], in1=bt[1][:],
                                        op=Alu.add)
                nc.gpsimd.tensor_tensor(out=t2[:], in0=t1[:], in1=bt[2][:],
                                        op=Alu.add)
                nc.gpsimd.tensor_tensor(out=t1[:], in0=t2[:], in1=iota_f[:],
                                        op=Alu.add)
                # sentinel select: kf = (key'-SENT)*hitm [DVE]; +SENT in cast
                nc.vector.scalar_tensor_tensor(out=t2[:], in0=t1[:],
                                               scalar=float(SENT),
                                               in1=hitm[:], op0=Alu.subtract,
                                               op1=Alu.mult)

                A = pool.tile(big, bf16, name=f"A{t}", tag=f"A{t}")
                B = pool.tile(big, bf16, name=f"B{t}", tag=f"B{t}")
                # cast (kf + SENT) fp32 -> uint16 bit patterns, into A
                nc.scalar.activation(A[:].bitcast(u16), t2[:], Act.Copy,
                                     bias=float(SENT))

                # ---- phase 2: bitonic top-256, rotated layout [DVE] ----
                # physical col = q*8 + c ;  c = chunk id (never compared)
                cur, other = A, B

                def substage(lo_in, hi_in, lo_out, hi_out):
                    nonlocal cur, other
                    nc.vector.tensor_tensor(out=lo_out, in0=lo_in, in1=hi_in,
                                            op=Alu.min)
                    nc.vector.tensor_tensor(out=hi_out, in0=lo_in, in1=hi_in,
                                            op=Alu.max)
                    cur, other = other, cur

                def dist_substage(d, nch):
                    # pairs (q, q+d) over q-range, chunks 0..nch-1 innermost
                    r_in = cur[:].rearrange("p (b r c) -> p b r c",
                                            r=2 * d, c=NCH)
                    r_out = other[:].rearrange("p (b r c) -> p b r c",
                                               r=2 * d, c=NCH)
                    substage(r_in[:, :, 0:d, 0:nch], r_in[:, :, d:2 * d, 0:nch],
                             r_out[:, :, 0:d, 0:nch], r_out[:, :, d:2 * d, 0:nch])

                # stage A: sort each 256-chunk ascending (reversed-read merges)
                for mexp in range(8):
                    m = 1 << mexp
                    r_in = cur[:].rearrange("p (b r c) -> p b r c",
                                            r=2 * m, c=NCH)
                    r_out = other[:].rearrange("p (b r c) -> p b r c",
                                               r=2 * m, c=NCH)
                    substage(r_in[:, :, 0:m, :],
                             r_in[:, :, m:2 * m, :][:, :, ::-1, :],
                             r_out[:, :, 0:m, :],
                             r_out[:, :, m:2 * m, :][:, :, ::-1, :])
                    d = m // 2
                    while d >= 1:
                        dist_substage(d, NCH)
                        d //= 2

                # stage B: prune-merges 8 -> 4 -> 2 -> 1 lists of 256
                nch = NCH
                while nch > 1:
                    half = nch // 2
                    r_in = cur[:].rearrange("p (q c) -> p q c", c=NCH)
                    r_out = other[:].rearrange("p (q c) -> p q c", c=NCH)
                    # list l = min(chunk 2l [q], chunk 2l+1 [255-q])
                    nc.vector.tensor_tensor(
                        out=r_out[:, :, 0:half],
                        in0=r_in[:, :, 0:nch:2],
                        in1=r_in[:, ::-1, 1:nch:2],
                        op=Alu.min)
                    cur, other = other, cur
                    d = CHUNK // 2
                    while d >= 1:
                        dist_substage(d, half)
                        d //= 2
                    nch = half

                out_view = cur[:].rearrange("p (q c) -> p q c", c=NCH)
                nc.sync.dma_start(out=keys_d[t],
                                  in_=out_view[:, :, 0].bitcast(u16))

    nc.compile()
    return nc


def _get_program():
    if "nc" not in _cached:
        _cached["nc"] = _build_program()
    return _cached["nc"]


_PERM = ((np.arange(N) % NCH) * CHUNK + np.arange(N) // NCH)  # logical j at phys col


def _make_in_maps(pos):
    pjb = np.ascontiguousarray(pos.T[:, _PERM])  # [3, N], rotated columns
    in_maps = []
    for cr in range(NCORES):
        rows0 = cr * ROWS_PER_CORE
        pit = pos[rows0: rows0 + ROWS_PER_CORE].reshape(NTILES, 128, 3)
        nig = -(OFF + rows0 + np.arange(ROWS_PER_CORE, dtype=np.float32)
                ).reshape(NTILES, 128, 1).astype(np.float32)
        nfl = -(pit > 0.5).astype(np.float32)
        in_maps.append({
            "pjb": pjb,
            "npi": np.ascontiguousarray(-pit),
            "nig": np.ascontiguousarray(nig),
            "nfl": np.ascontiguousarray(nfl),
        })
    return in_maps


def kernel(positions, cell, max_neighbours):
    from concourse.bass_utils import run_bass_kernel_spmd

    pos = np.asarray(positions, dtype=np.float32)
    assert pos.shape == (N, 3)
    k = int(max_neighbours)
    assert k == K, f"kernel hardcodes max_neighbours=256, got {k}"

    nc = _get_program()
    res = run_bass_kernel_spmd(nc, _make_in_maps(pos),
                               core_ids=list(range(NCORES)))

    keys = np.concatenate(
        [r["keys"].reshape(ROWS_PER_CORE, K) for r in res.results], axis=0)
    counts = np.concatenate(
        [r["cnt"].reshape(ROWS_PER_CORE) for r in res.results], axis=0)

    raw = keys.astype(np.int64)
    valid = raw != SENT
    key = raw - OFF
    j = key & (N - 1)
    cp = key >> 11
    bx = cp & 1
    by = (cp >> 1) & 1
    bz = cp >> 2
    f = pos > 0.5  # same rule as device flip2
    g = np.empty((N, K, 3), np.int64)
    for a, bbit in enumerate((bx, by, bz)):
        fa = f[:, a][:, None]
        g[:, :, a] = np.where(fa, np.where(bbit > 0, 1, 0),
                              np.where(bbit > 0, 0, -1))
    neighbours = np.where(valid, j, -1).astype(np.int32)
    cells = np.where(valid[..., None], g, 1).astype(np.int32)
    actual_max = np.int32(counts.max())
    return neighbours, cells, actual_max


# revision 8
# speedup vs baseline: 2.9189x; 2.9189x over previous
"""Periodic-boundary fixed-capacity neighbour list on 8 trn2 NeuronCores.

Device algorithm (per core, 256 rows as 2 partition-tiles):
  For unit cell + cutoff 0.3, a pair (i, j) is within cutoff for at most ONE
  of the 27 periodic images, and per axis at most TWO image shifts are ever
  feasible for a given centre row ({0,+1} if p_i>0.5 else {-1,0}).  So the
  N x 27N reference mask collapses to N x N with a 3-bit reduced cell code:
     v_a = p_j,a - p_i,a              (ACT FMA, bit-exact)
     g_a = -sign(v_a) * [|v_a| > 0.5]   (epsilon-boundary differences are
                                         no-hit safe: |w| ~ 0.5 >> 0.3)
     u_a = g_a + p_j,a ;  w_a = u_a - p_i,a    (fp32, reference-exact order)
     hit = ((wx^2+wy^2)+wz^2) <= 0.09f  and  j != i
     b_a = [u_a - flip2_a >= 0]   (exact: Sterbenz),  flip2_a = [p_i,a>0.5]
     key = (bz*4+by*2+bx)*2048 + j + 8192  in [8192, 24576) ; miss -> 32512
  Ascending-key order == the reference argwhere packing order.  Keys are
  cast to uint16 and REINTERPRETED as bf16 (positive-float bit patterns
  sort like integers), so the bitonic top-256 (sort eight 256-chunks, then
  reversed-read prune-merges 8->4->2->1) runs mostly in the DVE 2x 16-bit
  mode.  Phase 1 is spread over ACT (FMA/Square/Abs/Sign, bitwise-IEEE
  verified), GPSIMD (tensor-tensor add/sub/mult only) and DVE immediate-
  scalar compares; both tiles' phase 1 is emitted before the two sorts so
  GPSIMD/ACT overlap the DVE sort of the previous tile.
Host: shard/replicate inputs, decode keys -> neighbours/cell_indices
(b-bit + flip rule -> shift vector), max of per-row hit counts ->
actual_max.  jnp.take(idx=-1) wraps: invalid cell slots = shifts[26] =
(1,1,1).
"""
import sys

if '/opt/trn_rl_repo' not in sys.path:
    sys.path.insert(0, '/opt/trn_rl_repo')

import numpy as np

N = 2048
K = 256
CHUNK = 256
NCORES = 8
ROWS_PER_CORE = N // NCORES  # 256
NTILES = ROWS_PER_CORE // 128  # 2
OFF = 8192
SENT = 32512  # 0x7F00 as uint16; huge finite positive as bf16
THR = 0.3 * 0.3  # fp32-converts to 0.090000004 like the jax reference

_cached = {}


def _build_program():
    import concourse.bacc as bacc
    import concourse.mybir as mybir
    from concourse.tile import TileContext

    f32 = mybir.dt.float32
    i32 = mybir.dt.int32
    u8 = mybir.dt.uint8
    u16 = mybir.dt.uint16
    bf16 = mybir.dt.bfloat16
    Alu = mybir.AluOpType
    Act = mybir.ActivationFunctionType

    nc = bacc.Bacc("TRN2", target_bir_lowering=False)

    pjb_d = nc.dram_tensor("pjb", [3, N], f32, kind="ExternalInput")
    npi_d = nc.dram_tensor("npi", [NTILES, 128, 3], f32, kind="ExternalInput")
    nig_d = nc.dram_tensor("nig", [NTILES, 128, 1], f32, kind="ExternalInput")
    nfl_d = nc.dram_tensor("nfl", [NTILES, 128, 3], f32, kind="ExternalInput")
    keys_d = nc.dram_tensor("keys", [NTILES, 128, K], u16, kind="ExternalOutput")
    cnt_d = nc.dram_tensor("cnt", [NTILES, 128, 1], f32, kind="ExternalOutput")

    with TileContext(nc) as tc:
        with tc.tile_pool(name="main", bufs=1) as pool:
            big = [128, N]
            pj = [pool.tile(big, f32, name=f"pj{a}", tag=f"pj{a}") for a in range(3)]
            iota_i = pool.tile(big, i32, tag="iotai")
            iota_f = pool.tile(big, f32, tag="iotaf")

            for a in range(3):
                nc.sync.dma_start(
                    out=pj[a][:],
                    in_=pjb_d[a:a + 1, :].partition_broadcast(128).squeeze(1),
                )
            nc.gpsimd.iota(iota_i[:], pattern=[[1, N]], base=OFF,
                           channel_multiplier=0)
            nc.vector.tensor_copy(out=iota_f[:], in_=iota_i[:])

            AB = []
            for t in range(NTILES):
                AB.append((pool.tile(big, bf16, name=f"A{t}", tag=f"A{t}"),
                           pool.tile(big, bf16, name=f"B{t}", tag=f"B{t}")))

            def phase1(t):
                npi_s = pool.tile([128, 3], f32, tag=f"npi{t}")
                nig_s = pool.tile([128, 1], f32, tag=f"nig{t}")
                nfl_s = pool.tile([128, 3], f32, tag=f"nfl{t}")
                cnt_s = pool.tile([128, 1], f32, tag=f"cnt{t}")
                nc.sync.dma_start(out=npi_s[:], in_=npi_d[t])
                nc.sync.dma_start(out=nig_s[:], in_=nig_d[t])
                nc.sync.dma_start(out=nfl_s[:], in_=nfl_d[t])

                u = [pool.tile(big, f32, name=f"u{a}", tag=f"u{a}")
                     for a in range(3)]
                bt = [pool.tile(big, f32, name=f"bt{a}", tag=f"bt{a}")
                      for a in range(3)]
                t1 = pool.tile(big, f32, tag="t1")
                t2 = pool.tile(big, f32, tag="t2")
                d2 = pool.tile(big, f32, tag="d2")
                hitm = pool.tile(big, f32, tag="hitm")
                noteq = pool.tile(big, f32, tag="noteq")
                dump = pool.tile(big, u8, tag="dump")

                for a in range(3):
                    va = pool.tile(big, f32, name=f"va{a}", tag="va")
                    av = pool.tile(big, f32, name=f"av{a}", tag="av")
                    sgn = pool.tile(big, f32, name=f"sgn{a}", tag="sgn")
                    uba = pool.tile(big, f32, name=f"uba{a}", tag="uba")
                    # v = pj - pi ; |v| ; sign(v)   [ACT, bit-exact]
                    nc.scalar.activation(va[:], pj[a][:], Act.Identity,
                                         bias=npi_s[:, a:a + 1], scale=1.0)
                    nc.scalar.activation(av[:], va[:], Act.Abs)
                    nc.scalar.activation(sgn[:], va[:], Act.Sign)
                    # big = [|v| > 0.5]  [DVE imm]
                    nc.vector.tensor_scalar(out=t1[:], in0=av[:],
                                            scalar1=0.5, scalar2=None,
                                            op0=Alu.is_gt)
                    # g = -sign*big ; u = pj - sign*big  [GPSIMD]
                    nc.gpsimd.tensor_tensor(out=t2[:], in0=sgn[:], in1=t1[:],
                                            op=Alu.mult)
                    nc.gpsimd.tensor_tensor(out=u[a][:], in0=pj[a][:],
                                            in1=t2[:], op=Alu.subtract)
                    # order bit: (u - flip2 >= 0) * 2^(11+a)
                    nc.scalar.activation(uba[:], u[a][:], Act.Identity,
                                         bias=nfl_s[:, a:a + 1], scale=1.0)
                    nc.vector.tensor_scalar(out=bt[a][:], in0=uba[:],
                                            scalar1=0.0,
                                            scalar2=float(2048 << a),
                                            op0=Alu.is_ge, op1=Alu.mult)
                    # w = u - pi ; sq = w^2  [ACT, bit-exact, in-place]
                    nc.scalar.activation(u[a][:], u[a][:], Act.Identity,
                                         bias=npi_s[:, a:a + 1], scale=1.0)
                    nc.scalar.activation(u[a][:], u[a][:], Act.Square)

                # d2 = (sq0 + sq1) + sq2   [GPSIMD]
                nc.gpsimd.tensor_tensor(out=t2[:], in0=u[0][:], in1=u[1][:],
                                        op=Alu.add)
                nc.gpsimd.tensor_tensor(out=d2[:], in0=t2[:], in1=u[2][:],
                                        op=Alu.add)
                # hit = d2 <= THR [DVE imm]
                nc.vector.tensor_scalar(out=t2[:], in0=d2[:], scalar1=THR,
                                        scalar2=None, op0=Alu.is_le)
                # self-exclusion: iota != ig via ACT diff + imm compare
                vn = pool.tile(big, f32, name=f"vn{t}", tag="va")
                nc.scalar.activation(vn[:], iota_f[:], Act.Identity,
                                     bias=nig_s[:], scale=1.0)
                nc.vector.tensor_scalar(out=noteq[:], in0=vn[:], scalar1=0.0,
                                        scalar2=None, op0=Alu.not_equal)
                nc.gpsimd.tensor_tensor(out=hitm[:], in0=t2[:], in1=noteq[:],
                                        op=Alu.mult)
                # per-row count via ACT accumulate (sum of 0/1)
                nc.scalar.activation(dump[:], hitm[:], Act.Copy,
                                     accum_out=cnt_s[:])
                nc.sync.dma_start(out=cnt_d[t], in_=cnt_s[:])

                # key' = bx' + by' + bz' + (8192 + j)    [GPSIMD adds]
                nc.gpsimd.tensor_tensor(out=t1[:], in0=bt[0][:], in1=bt[1][:],
                                        op=Alu.add)
                nc.gpsimd.tensor_tensor(out=t2[:], in0=t1[:], in1=bt[2][:],
                                        op=Alu.add)
                nc.gpsimd.tensor_tensor(out=t1[:], in0=t2[:], in1=iota_f[:],
                                        op=Alu.add)
                # sentinel select: kf = (key'-SENT)*hitm [DVE]; +SENT in cast
                nc.vector.scalar_tensor_tensor(out=t2[:], in0=t1[:],
                                               scalar=float(SENT),
                                               in1=hitm[:], op0=Alu.subtract,
                                               op1=Alu.mult)
                A = AB[t][0]
                nc.scalar.activation(A[:].bitcast(u16), t2[:], Act.Copy,
                                     bias=float(SENT))

            def phase2(t):
                cur, other = AB[t]

                def substage(lo_in, hi_in, lo_out, hi_out):
                    nonlocal cur, other
                    nc.vector.tensor_tensor(out=lo_out, in0=lo_in, in1=hi_in,
                                            op=Alu.min)
                    nc.vector.tensor_tensor(out=hi_out, in0=lo_in, in1=hi_in,
                                            op=Alu.max)
                    cur, other = other, cur

                def dist_substage(width, d):
                    r_in = cur[:, :width].rearrange("p (b r) -> p b r", r=2 * d)
                    r_out = other[:, :width].rearrange("p (b r) -> p b r",
                                                       r=2 * d)
                    substage(r_in[:, :, 0:d], r_in[:, :, d:2 * d],
                             r_out[:, :, 0:d], r_out[:, :, d:2 * d])

                # sort each 256-chunk ascending (reversed-read merges)
                for mexp in range(8):
                    m = 1 << mexp
                    r_in = cur[:].rearrange("p (b r) -> p b r", r=2 * m)
                    r_out = other[:].rearrange("p (b r) -> p b r", r=2 * m)
                    substage(r_in[:, :, 0:m], r_in[:, :, m:2 * m][:, :, ::-1],
                             r_out[:, :, 0:m], r_out[:, :, m:2 * m][:, :, ::-1])
                    d = m // 2
                    while d >= 1:
                        dist_substage(N, d)
                        d //= 2

                # prune-merges 8 -> 4 -> 2 -> 1 lists of 256
                width = N
                while width > CHUNK:
                    half = width // 2
                    r_in = cur[:, :width].rearrange("p (l r) -> p l r",
                                                    r=2 * CHUNK)
                    r_out = other[:, :half].rearrange("p (l r) -> p l r",
                                                      r=CHUNK)
                    nc.vector.tensor_tensor(
                        out=r_out[:],
                        in0=r_in[:, :, 0:CHUNK],
                        in1=r_in[:, :, CHUNK:2 * CHUNK][:, :, ::-1],
                        op=Alu.min)
                    cur, other = other, cur
                    d = CHUNK // 2
                    while d >= 1:
                        dist_substage(half, d)
                        d //= 2
                    width = half

                nc.sync.dma_start(out=keys_d[t], in_=cur[:, :K].bitcast(u16))

            for t in range(NTILES):
                phase1(t)
            for t in range(NTILES):
                phase2(t)

    nc.compile()
    return nc


def _get_program():
    if "nc" not in _cached:
        _cached["nc"] = _build_program()
    return _cached["nc"]


def _make_in_maps(pos):
    pjb = np.ascontiguousarray(pos.T)  # [3, N]
    in_maps = []
    for cr in range(NCORES):
        rows0 = cr * ROWS_PER_CORE
        pit = pos[rows0: rows0 + ROWS_PER_CORE].reshape(NTILES, 128, 3)
        nig = -(OFF + rows0 + np.arange(ROWS_PER_CORE, dtype=np.float32)
                ).reshape(NTILES, 128, 1).astype(np.float32)
        nfl = -(pit > 0.5).astype(np.float32)
        in_maps.append({
            "pjb": pjb,
            "npi": np.ascontiguousarray(-pit),
            "nig": np.ascontiguousarray(nig),
            "nfl": np.ascontiguousarray(nfl),
        })
    return in_maps


def kernel(positions, cell, max_neighbours):
    from concourse.bass_utils import run_bass_kernel_spmd

    pos = np.asarray(positions, dtype=np.float32)
    assert pos.shape == (N, 3)
    k = int(max_neighbours)
    assert k == K, f"kernel hardcodes max_neighbours=256, got {k}"

    nc = _get_program()
    res = run_bass_kernel_spmd(nc, _make_in_maps(pos),
                               core_ids=list(range(NCORES)))

    keys = np.concatenate(
        [r["keys"].reshape(ROWS_PER_CORE, K) for r in res.results], axis=0)
    counts = np.concatenate(
        [r["cnt"].reshape(ROWS_PER_CORE) for r in res.results], axis=0)

    raw = keys.astype(np.int64)
    valid = raw != SENT
    key = raw - OFF
    j = key & (N - 1)
    cp = key >> 11
    bx = cp & 1
    by = (cp >> 1) & 1
    bz = cp >> 2
    f = pos > 0.5  # same rule as device flip2
    g = np.empty((N, K, 3), np.int64)
    for a, bbit in enumerate((bx, by, bz)):
        fa = f[:, a][:, None]
        g[:, :, a] = np.where(fa, np.where(bbit > 0, 1, 0),
                              np.where(bbit > 0, 0, -1))
    neighbours = np.where(valid, j, -1).astype(np.int32)
    cells = np.where(valid[..., None], g, 1).astype(np.int32)
    actual_max = np.int32(counts.max())
    return neighbours, cells, actual_max


# revision 9
# speedup vs baseline: 3.0398x; 1.0414x over previous
"""Periodic-boundary fixed-capacity neighbour list on 8 trn2 NeuronCores.

Device algorithm (per core, 256 rows as 2 partition-tiles):
  For unit cell + cutoff 0.3, a pair (i, j) is within cutoff for at most ONE
  of the 27 periodic images, and per axis at most TWO image shifts are ever
  feasible for a given centre row ({0,+1} if p_i>0.5 else {-1,0}).  So the
  N x 27N reference mask collapses to N x N with a 3-bit reduced cell code:
     v_a = p_j,a - p_i,a          (ACT FMA; compare vs +-0.5 immediates)
     g_a = [v<-0.5] - [v>0.5]     (threshold-epsilon differences are no-hit
                                   safe: |w| would be ~0.5 >> 0.3)
     u_a = g_a + p_j,a ;  w_a = u_a - p_i,a    (fp32, reference-exact order)
     hit = ((wx^2+wy^2)+wz^2) <= 0.09f  and  j != i
     b_a = [u_a - flip2_a >= 0]   (exact: Sterbenz),  flip2_a = [p_i,a>0.5]
     key = (bz*4+by*2+bx)*2048 + j + 8192  in [8192, 24576) ; miss -> 32512
  Ascending-key order == the reference argwhere packing order.  Keys are
  cast to uint16 and REINTERPRETED as bf16 (positive-float bit patterns
  sort like integers), so the bitonic top-256 (sort eight 256-chunks, then
  reversed-read prune-merges 8->4->2->1) runs mostly in the DVE 2x 16-bit
  mode.  Phase-1 is spread over ACT (FMA/Square, bitwise-IEEE verified),
  GPSIMD (tensor-tensor add/sub/mult only) and DVE immediate-scalar
  compares, so the DVE mostly runs the sort network.
Host: shard/replicate inputs, decode keys -> neighbours/cell_indices
(b-bit + flip rule -> shift vector), max of per-row hit counts ->
actual_max.  jnp.take(idx=-1) wraps: invalid cell slots = shifts[26] =
(1,1,1).
"""
import sys

if '/opt/trn_rl_repo' not in sys.path:
    sys.path.insert(0, '/opt/trn_rl_repo')

import numpy as np

N = 2048
K = 256
CHUNK = 256
NCORES = 8
ROWS_PER_CORE = N // NCORES  # 256
NTILES = ROWS_PER_CORE // 128  # 2
OFF = 8192
SENT = 32512  # 0x7F00 as uint16; huge finite positive as bf16
THR = 0.3 * 0.3  # fp32-converts to 0.090000004 like the jax reference

_cached = {}


def _build_program():
    import concourse.bacc as bacc
    import concourse.mybir as mybir
    from concourse.tile import TileContext

    f32 = mybir.dt.float32
    i32 = mybir.dt.int32
    u16 = mybir.dt.uint16
    bf16 = mybir.dt.bfloat16
    Alu = mybir.AluOpType
    Act = mybir.ActivationFunctionType
    Ax = mybir.AxisListType

    nc = bacc.Bacc("TRN2", target_bir_lowering=False)

    pjb_d = nc.dram_tensor("pjb", [3, N], f32, kind="ExternalInput")
    pit_d = nc.dram_tensor("pit", [NTILES, 128, 3], f32, kind="ExternalInput")
    ig_d = nc.dram_tensor("ig", [NTILES, 128, 1], f32, kind="ExternalInput")
    fl_d = nc.dram_tensor("fl", [NTILES, 128, 3], f32, kind="ExternalInput")
    keys_d = nc.dram_tensor("keys", [NTILES, 128, K], u16, kind="ExternalOutput")
    cnt_d = nc.dram_tensor("cnt", [NTILES, 128, 1], f32, kind="ExternalOutput")

    with TileContext(nc) as tc:
        with tc.tile_pool(name="main", bufs=1) as pool:
            big = [128, N]
            pj = [pool.tile(big, f32, name=f"pj{a}", tag=f"pj{a}") for a in range(3)]
            iota_i = pool.tile(big, i32, tag="iotai")
            iota_f = pool.tile(big, f32, tag="iotaf")

            for a in range(3):
                nc.sync.dma_start(
                    out=pj[a][:],
                    in_=pjb_d[a:a + 1, :].partition_broadcast(128).squeeze(1),
                )
            nc.gpsimd.iota(iota_i[:], pattern=[[1, N]], base=OFF,
                           channel_multiplier=0)
            nc.vector.tensor_copy(out=iota_f[:], in_=iota_i[:])

            for t in range(NTILES):
                pit_s = pool.tile([128, 3], f32, tag=f"pit{t}")
                nig_s = pool.tile([128, 1], f32, tag=f"nig{t}")
                nfl_s = pool.tile([128, 3], f32, tag=f"nfl{t}")
                npi_s = pool.tile([128, 3], f32, tag=f"npi{t}")
                cnt_s = pool.tile([128, 1], f32, tag=f"cnt{t}")
                nc.sync.dma_start(out=pit_s[:], in_=pit_d[t])
                ig_s = pool.tile([128, 1], f32, tag=f"ig{t}")
                fl_s = pool.tile([128, 3], f32, tag=f"fl{t}")
                nc.sync.dma_start(out=ig_s[:], in_=ig_d[t])
                nc.sync.dma_start(out=fl_s[:], in_=fl_d[t])
                nc.vector.tensor_scalar(out=npi_s[:], in0=pit_s[:], scalar1=-1.0,
                                        scalar2=None, op0=Alu.mult)
                nc.vector.tensor_scalar(out=nig_s[:], in0=ig_s[:], scalar1=-1.0,
                                        scalar2=None, op0=Alu.mult)
                nc.vector.tensor_scalar(out=nfl_s[:], in0=fl_s[:], scalar1=-1.0,
                                        scalar2=None, op0=Alu.mult)

                u = [pool.tile(big, f32, name=f"u{a}", tag=f"u{a}") for a in range(3)]
                bt = [pool.tile(big, f32, name=f"bt{a}", tag=f"bt{a}") for a in range(3)]
                t1 = pool.tile(big, f32, tag="t1")
                t2 = pool.tile(big, f32, tag="t2")
                d2 = pool.tile(big, f32, tag="d2")
                hitm = pool.tile(big, f32, tag="hitm")
                noteq = pool.tile(big, f32, tag="noteq")

                for a in range(3):
                    va = pool.tile(big, f32, name=f"va{a}", tag="va")
                    cma = pool.tile(big, f32, name=f"cma{a}", tag="cma")
                    cmb = pool.tile(big, f32, name=f"cmb{a}", tag="cmb")
                    uba = pool.tile(big, f32, name=f"uba{a}", tag="uba")
                    # v = pj - pi  [ACT FMA, bit-exact]
                    nc.scalar.activation(va[:], pj[a][:], Act.Identity,
                                         bias=npi_s[:, a:a + 1], scale=1.0)
                    # g parts via immediate compares [DVE 2x]
                    nc.vector.tensor_scalar(out=cma[:], in0=va[:],
                                            scalar1=-0.5, scalar2=None,
                                            op0=Alu.is_lt)
                    nc.vector.tensor_scalar(out=cmb[:], in0=va[:],
                                            scalar1=0.5, scalar2=None,
                                            op0=Alu.is_gt)
                    # g = cma - cmb ; u = g + pj   [GPSIMD]
                    nc.gpsimd.tensor_tensor(out=t1[:], in0=cma[:], in1=cmb[:],
                                            op=Alu.subtract)
                    nc.gpsimd.tensor_tensor(out=u[a][:], in0=t1[:],
                                            in1=pj[a][:], op=Alu.add)
                    # order bit: (u - flip2 >= 0) * 2^(11+a)
                    nc.scalar.activation(uba[:], u[a][:], Act.Identity,
                                         bias=nfl_s[:, a:a + 1], scale=1.0)
                    nc.vector.tensor_scalar(out=bt[a][:], in0=uba[:],
                                            scalar1=0.0,
                                            scalar2=float(2048 << a),
                                            op0=Alu.is_ge, op1=Alu.mult)
                    # w = u - pi ; sq = w^2  [ACT, bit-exact, in-place]
                    nc.scalar.activation(u[a][:], u[a][:], Act.Identity,
                                         bias=npi_s[:, a:a + 1], scale=1.0)
                    nc.scalar.activation(u[a][:], u[a][:], Act.Square)

                # d2 = (sq0 + sq1) + sq2   [DVE]
                nc.vector.tensor_tensor(out=t2[:], in0=u[0][:], in1=u[1][:],
                                        op=Alu.add)
                nc.vector.tensor_tensor(out=d2[:], in0=t2[:], in1=u[2][:],
                                        op=Alu.add)
                # hit = d2 <= THR [DVE imm]
                nc.vector.tensor_scalar(out=t2[:], in0=d2[:], scalar1=THR,
                                        scalar2=None, op0=Alu.is_le)
                # self-exclusion: iota != ig via ACT diff + imm compare
                vn = pool.tile(big, f32, name=f"vn{t}", tag="va")
                nc.scalar.activation(vn[:], iota_f[:], Act.Identity,
                                     bias=nig_s[:], scale=1.0)
                nc.vector.tensor_scalar(out=noteq[:], in0=vn[:], scalar1=0.0,
                                        scalar2=None, op0=Alu.not_equal)
                nc.gpsimd.tensor_tensor(out=hitm[:], in0=t2[:], in1=noteq[:],
                                        op=Alu.mult)
                nc.vector.tensor_reduce(out=cnt_s[:], in_=hitm[:], axis=Ax.X,
                                        op=Alu.add)
                nc.sync.dma_start(out=cnt_d[t], in_=cnt_s[:])

                # key' = bx' + by' + bz' + (8192 + j)    [GPSIMD adds]
                nc.gpsimd.tensor_tensor(out=t1[:], in0=bt[0][:], in1=bt[1][:],
                                        op=Alu.add)
                nc.gpsimd.tensor_tensor(out=t2[:], in0=t1[:], in1=bt[2][:],
                                        op=Alu.add)
                nc.gpsimd.tensor_tensor(out=t1[:], in0=t2[:], in1=iota_f[:],
                                        op=Alu.add)
                # sentinel select: kf = (key'-SENT)*hitm + SENT  [DVE, exact]
                nc.vector.scalar_tensor_tensor(out=t2[:], in0=t1[:],
                                               scalar=float(SENT),
                                               in1=hitm[:], op0=Alu.subtract,
                                               op1=Alu.mult)
                nc.vector.tensor_scalar(out=t1[:], in0=t2[:],
                                        scalar1=float(SENT), scalar2=None,
                                        op0=Alu.add)

                A = pool.tile(big, bf16, name=f"A{t}", tag=f"A{t}")
                B = pool.tile(big, bf16, name=f"B{t}", tag=f"B{t}")
                # cast fp32 int-valued -> uint16 bit patterns, into A
                nc.scalar.activation(A[:].bitcast(u16), t1[:], Act.Copy)

                # ---- phase 2: bitonic top-256 on bf16 bit patterns [DVE] ----
                cur, other = A, B

                def substage(lo_in, hi_in, lo_out, hi_out):
                    nonlocal cur, other
                    nc.vector.tensor_tensor(out=lo_out, in0=lo_in, in1=hi_in,
                                            op=Alu.min)
                    nc.vector.tensor_tensor(out=hi_out, in0=lo_in, in1=hi_in,
                                            op=Alu.max)
                    cur, other = other, cur

                def dist_substage(width, d):
                    r_in = cur[:, :width].rearrange("p (b r) -> p b r", r=2 * d)
                    r_out = other[:, :width].rearrange("p (b r) -> p b r", r=2 * d)
                    substage(r_in[:, :, 0:d], r_in[:, :, d:2 * d],
                             r_out[:, :, 0:d], r_out[:, :, d:2 * d])

                # stage A: sort each 256-chunk ascending (reversed-read merges)
                for mexp in range(8):
                    m = 1 << mexp
                    r_in = cur[:].rearrange("p (b r) -> p b r", r=2 * m)
                    r_out = other[:].rearrange("p (b r) -> p b r", r=2 * m)
                    substage(r_in[:, :, 0:m], r_in[:, :, m:2 * m][:, :, ::-1],
                             r_out[:, :, 0:m], r_out[:, :, m:2 * m][:, :, ::-1])
                    d = m // 2
                    while d >= 1:
                        dist_substage(N, d)
                        d //= 2

                # stage B: prune-merges 8 -> 4 -> 2 -> 1 lists of 256
                width = N
                while width > CHUNK:
                    half = width // 2
                    r_in = cur[:, :width].rearrange("p (l r) -> p l r",
                                                    r=2 * CHUNK)
                    r_out = other[:, :half].rearrange("p (l r) -> p l r",
                                                      r=CHUNK)
                    nc.vector.tensor_tensor(
                        out=r_out[:],
                        in0=r_in[:, :, 0:CHUNK],
                        in1=r_in[:, :, CHUNK:2 * CHUNK][:, :, ::-1],
                        op=Alu.min)
                    cur, other = other, cur
                    d = CHUNK // 2
                    while d >= 1:
                        dist_substage(half, d)
                        d //= 2
                    width = half

                nc.sync.dma_start(out=keys_d[t], in_=cur[:, :K].bitcast(u16))

    nc.compile()
    return nc


def _get_program():
    if "nc" not in _cached:
        _cached["nc"] = _build_program()
    return _cached["nc"]


def _make_in_maps(pos):
    pjb = np.ascontiguousarray(pos.T)  # [3, N]
    in_maps = []
    for cr in range(NCORES):
        rows0 = cr * ROWS_PER_CORE
        pit = pos[rows0: rows0 + ROWS_PER_CORE].reshape(NTILES, 128, 3)
        ig = (OFF + rows0 + np.arange(ROWS_PER_CORE, dtype=np.float32)
              ).reshape(NTILES, 128, 1).astype(np.float32)
        fl = (pit > 0.5).astype(np.float32)
        in_maps.append({
            "pjb": pjb,
            "pit": np.ascontiguousarray(pit),
            "ig": np.ascontiguousarray(ig),
            "fl": np.ascontiguousarray(fl),
        })
    return in_maps


def kernel(positions, cell, max_neighbours):
    from concourse.bass_utils import run_bass_kernel_spmd

    pos = np.asarray(positions, dtype=np.float32)
    assert pos.shape == (N, 3)
    k = int(max_neighbours)
    assert k == K, f"kernel hardcodes max_neighbours=256, got {k}"

    nc = _get_program()
    res = run_bass_kernel_spmd(nc, _make_in_maps(pos),
                               core_ids=list(range(NCORES)))

    keys = np.concatenate(
        [r["keys"].reshape(ROWS_PER_CORE, K) for r in res.results], axis=0)
    counts = np.concatenate(
        [r["cnt"].reshape(ROWS_PER_CORE) for r in res.results], axis=0)

    raw = keys.astype(np.int64)
    valid = raw != SENT
    key = raw - OFF
    j = key & (N - 1)
    cp = key >> 11
    bx = cp & 1
    by = (cp >> 1) & 1
    bz = cp >> 2
    f = pos > 0.5  # same rule as device flip2
    g = np.empty((N, K, 3), np.int64)
    for a, bbit in enumerate((bx, by, bz)):
        fa = f[:, a][:, None]
        g[:, :, a] = np.where(fa, np.where(bbit > 0, 1, 0),
                              np.where(bbit > 0, 0, -1))
    neighbours = np.where(valid, j, -1).astype(np.int32)
    cells = np.where(valid[..., None], g, 1).astype(np.int32)
    actual_max = np.int32(counts.max())
    return neighbours, cells, actual_max


# revision 11
# speedup vs baseline: 3.1232x; 1.0274x over previous
"""Periodic-boundary fixed-capacity neighbour list on 8 trn2 NeuronCores.

Device algorithm (per core, 256 rows as 2 partition-tiles):
  For unit cell + cutoff 0.3, a pair (i, j) is within cutoff for at most ONE
  of the 27 periodic images, and per axis at most TWO image shifts are ever
  feasible for a given centre row ({0,+1} if p_i>0.5 else {-1,0}).  So the
  N x 27N reference mask collapses to N x N with a 3-bit reduced cell code:
     v_a = p_j,a - p_i,a          (ACT FMA; compare vs +-0.5 immediates)
     g_a = [v<-0.5] - [v>0.5]     (threshold-epsilon differences are no-hit
                                   safe: |w| would be ~0.5 >> 0.3)
     u_a = g_a + p_j,a ;  w_a = u_a - p_i,a    (fp32, reference-exact order)
     hit = ((wx^2+wy^2)+wz^2) <= 0.09f  and  j != i
     b_a = [u_a - flip2_a >= 0]   (exact: Sterbenz),  flip2_a = [p_i,a>0.5]
     key = (bz*4+by*2+bx)*2048 + j + 8192  in [8192, 24576) ; miss -> 32512
  Ascending-key order == the reference argwhere packing order.  Keys are
  cast to uint16 and REINTERPRETED as bf16 (positive-float bit patterns
  sort like integers), so the bitonic top-256 (sort eight 256-chunks, then
  reversed-read prune-merges 8->4->2->1) runs mostly in the DVE 2x 16-bit
  mode.  Phase-1 is spread over ACT (FMA/Square, bitwise-IEEE verified),
  GPSIMD (tensor-tensor add/sub/mult only) and DVE immediate-scalar
  compares, so the DVE mostly runs the sort network.
Host: shard/replicate inputs, decode keys -> neighbours/cell_indices
(b-bit + flip rule -> shift vector), max of per-row hit counts ->
actual_max.  jnp.take(idx=-1) wraps: invalid cell slots = shifts[26] =
(1,1,1).
"""
import sys

if '/opt/trn_rl_repo' not in sys.path:
    sys.path.insert(0, '/opt/trn_rl_repo')

import numpy as np

N = 2048
K = 256
CHUNK = 256
NCORES = 8
ROWS_PER_CORE = N // NCORES  # 256
NTILES = ROWS_PER_CORE // 128  # 2
OFF = 8192
SENT = 32512  # 0x7F00 as uint16; huge finite positive as bf16
THR = 0.3 * 0.3  # fp32-converts to 0.090000004 like the jax reference

_cached = {}


def _build_program():
    import concourse.bacc as bacc
    import concourse.mybir as mybir
    from concourse.tile import TileContext

    f32 = mybir.dt.float32
    i32 = mybir.dt.int32
    u16 = mybir.dt.uint16
    bf16 = mybir.dt.bfloat16
    Alu = mybir.AluOpType
    Act = mybir.ActivationFunctionType
    Ax = mybir.AxisListType

    nc = bacc.Bacc("TRN2", target_bir_lowering=False)

    pjb_d = nc.dram_tensor("pjb", [3, N], f32, kind="ExternalInput")
    pit_d = nc.dram_tensor("pit", [NTILES, 128, 3], f32, kind="ExternalInput")
    ig_d = nc.dram_tensor("ig", [NTILES, 128, 1], f32, kind="ExternalInput")
    fl_d = nc.dram_tensor("fl", [NTILES, 128, 3], f32, kind="ExternalInput")
    keys_d = nc.dram_tensor("keys", [NTILES, 128, K], u16, kind="ExternalOutput")
    cnt_d = nc.dram_tensor("cnt", [NTILES, 128, 1], f32, kind="ExternalOutput")

    with TileContext(nc) as tc:
        with tc.tile_pool(name="main", bufs=1) as pool:
            big = [128, N]
            pj = [pool.tile(big, f32, name=f"pj{a}", tag=f"pj{a}") for a in range(3)]
            iota_i = pool.tile(big, i32, tag="iotai")
            iota_f = pool.tile(big, f32, tag="iotaf")

            for a in range(3):
                nc.sync.dma_start(
                    out=pj[a][:],
                    in_=pjb_d[a:a + 1, :].partition_broadcast(128).squeeze(1),
                )
            nc.gpsimd.iota(iota_i[:], pattern=[[1, N]], base=OFF,
                           channel_multiplier=0)
            nc.vector.tensor_copy(out=iota_f[:], in_=iota_i[:])

            for t in range(NTILES):
                pit_s = pool.tile([128, 3], f32, tag=f"pit{t}")
                nig_s = pool.tile([128, 1], f32, tag=f"nig{t}")
                nfl_s = pool.tile([128, 3], f32, tag=f"nfl{t}")
                npi_s = pool.tile([128, 3], f32, tag=f"npi{t}")
                cnt_s = pool.tile([128, 1], f32, tag=f"cnt{t}")
                nc.sync.dma_start(out=pit_s[:], in_=pit_d[t])
                ig_s = pool.tile([128, 1], f32, tag=f"ig{t}")
                fl_s = pool.tile([128, 3], f32, tag=f"fl{t}")
                nc.sync.dma_start(out=ig_s[:], in_=ig_d[t])
                nc.sync.dma_start(out=fl_s[:], in_=fl_d[t])
                nc.vector.tensor_scalar(out=npi_s[:], in0=pit_s[:], scalar1=-1.0,
                                        scalar2=None, op0=Alu.mult)
                nc.vector.tensor_scalar(out=nig_s[:], in0=ig_s[:], scalar1=-1.0,
                                        scalar2=None, op0=Alu.mult)
                nc.vector.tensor_scalar(out=nfl_s[:], in0=fl_s[:], scalar1=-1.0,
                                        scalar2=None, op0=Alu.mult)

                u = [pool.tile(big, f32, name=f"u{a}", tag=f"u{a}") for a in range(3)]
                bt = [pool.tile(big, f32, name=f"bt{a}", tag=f"bt{a}") for a in range(3)]
                t1 = pool.tile(big, f32, tag="t1")
                t2 = pool.tile(big, f32, tag="t2")
                d2 = pool.tile(big, f32, tag="d2")
                hitm = pool.tile(big, f32, tag="hitm")
                noteq = pool.tile(big, f32, tag="noteq")

                for a in range(3):
                    va = pool.tile(big, f32, name=f"va{a}", tag="va")
                    cma = pool.tile(big, f32, name=f"cma{a}", tag="cma")
                    cmb = pool.tile(big, f32, name=f"cmb{a}", tag="cmb")
                    uba = pool.tile(big, f32, name=f"uba{a}", tag="uba")
                    # v = pj - pi  [ACT FMA, bit-exact]
                    nc.scalar.activation(va[:], pj[a][:], Act.Identity,
                                         bias=npi_s[:, a:a + 1], scale=1.0)
                    # g parts via immediate compares [DVE 2x]
                    nc.vector.tensor_scalar(out=cma[:], in0=va[:],
                                            scalar1=-0.5, scalar2=None,
                                            op0=Alu.is_lt)
                    nc.vector.tensor_scalar(out=cmb[:], in0=va[:],
                                            scalar1=0.5, scalar2=None,
                                            op0=Alu.is_gt)
                    # g = cma - cmb ; u = g + pj   [GPSIMD]
                    nc.gpsimd.tensor_tensor(out=t1[:], in0=cma[:], in1=cmb[:],
                                            op=Alu.subtract)
                    nc.gpsimd.tensor_tensor(out=u[a][:], in0=t1[:],
                                            in1=pj[a][:], op=Alu.add)
                    # order bit: (u - flip2 >= 0) * 2^(11+a)
                    nc.scalar.activation(uba[:], u[a][:], Act.Identity,
                                         bias=nfl_s[:, a:a + 1], scale=1.0)
                    nc.vector.tensor_scalar(out=bt[a][:], in0=uba[:],
                                            scalar1=0.0,
                                            scalar2=float(2048 << a),
                                            op0=Alu.is_ge, op1=Alu.mult)
                    # w = u - pi ; sq = w^2  [ACT, bit-exact, in-place]
                    nc.scalar.activation(u[a][:], u[a][:], Act.Identity,
                                         bias=npi_s[:, a:a + 1], scale=1.0)
                    nc.scalar.activation(u[a][:], u[a][:], Act.Square)

                # d2 = (sq0 + sq1) + sq2   [DVE]
                nc.vector.tensor_tensor(out=t2[:], in0=u[0][:], in1=u[1][:],
                                        op=Alu.add)
                nc.vector.tensor_tensor(out=d2[:], in0=t2[:], in1=u[2][:],
                                        op=Alu.add)
                # hit = d2 <= THR [DVE imm]
                nc.vector.tensor_scalar(out=t2[:], in0=d2[:], scalar1=THR,
                                        scalar2=None, op0=Alu.is_le)
                # self-exclusion: iota != ig via ACT diff + imm compare
                vn = pool.tile(big, f32, name=f"vn{t}", tag="va")
                nc.scalar.activation(vn[:], iota_f[:], Act.Identity,
                                     bias=nig_s[:], scale=1.0)
                nc.vector.tensor_scalar(out=noteq[:], in0=vn[:], scalar1=0.0,
                                        scalar2=None, op0=Alu.not_equal)
                nc.vector.tensor_tensor(out=hitm[:], in0=t2[:], in1=noteq[:],
                                        op=Alu.mult)
                nc.vector.tensor_reduce(out=cnt_s[:], in_=hitm[:], axis=Ax.X,
                                        op=Alu.add)
                nc.sync.dma_start(out=cnt_d[t], in_=cnt_s[:])

                # key' = bx' + by' + bz' + (8192 + j)    [GPSIMD adds]
                nc.vector.tensor_tensor(out=t1[:], in0=bt[0][:], in1=bt[1][:],
                                        op=Alu.add)
                nc.vector.tensor_tensor(out=t2[:], in0=t1[:], in1=bt[2][:],
                                        op=Alu.add)
                nc.vector.tensor_tensor(out=t1[:], in0=t2[:], in1=iota_f[:],
                                        op=Alu.add)
                # sentinel select: kf = (key'-SENT)*hitm + SENT  [DVE, exact]
                nc.vector.scalar_tensor_tensor(out=t2[:], in0=t1[:],
                                               scalar=float(SENT),
                                               in1=hitm[:], op0=Alu.subtract,
                                               op1=Alu.mult)
                nc.vector.tensor_scalar(out=t1[:], in0=t2[:],
                                        scalar1=float(SENT), scalar2=None,
                                        op0=Alu.add)

                A = pool.tile(big, bf16, name=f"A{t}", tag=f"A{t}")
                B = pool.tile(big, bf16, name=f"B{t}", tag=f"B{t}")
                # cast fp32 int-valued -> uint16 bit patterns, into A
                nc.scalar.activation(A[:].bitcast(u16), t1[:], Act.Copy)

                # ---- phase 2: bitonic top-256 on bf16 bit patterns [DVE] ----
                cur, other = A, B

                def substage(lo_in, hi_in, lo_out, hi_out):
                    nonlocal cur, other
                    nc.vector.tensor_tensor(out=lo_out, in0=lo_in, in1=hi_in,
                                            op=Alu.min)
                    nc.vector.tensor_tensor(out=hi_out, in0=lo_in, in1=hi_in,
                                            op=Alu.max)
                    cur, other = other, cur

                def dist_substage(width, d):
                    r_in = cur[:, :width].rearrange("p (b r) -> p b r", r=2 * d)
                    r_out = other[:, :width].rearrange("p (b r) -> p b r", r=2 * d)
                    substage(r_in[:, :, 0:d], r_in[:, :, d:2 * d],
                             r_out[:, :, 0:d], r_out[:, :, d:2 * d])

                # stage A: sort each 256-chunk ascending (reversed-read merges)
                for mexp in range(8):
                    m = 1 << mexp
                    r_in = cur[:].rearrange("p (b r) -> p b r", r=2 * m)
                    r_out = other[:].rearrange("p (b r) -> p b r", r=2 * m)
                    substage(r_in[:, :, 0:m], r_in[:, :, m:2 * m][:, :, ::-1],
                             r_out[:, :, 0:m], r_out[:, :, m:2 * m][:, :, ::-1])
                    d = m // 2
                    while d >= 1:
                        dist_substage(N, d)
                        d //= 2

                # stage B: prune-merges 8 -> 4 -> 2 -> 1 lists of 256
                width = N
                while width > CHUNK:
                    half = width // 2
                    r_in = cur[:, :width].rearrange("p (l r) -> p l r",
                                                    r=2 * CHUNK)
                    r_out = other[:, :half].rearrange("p (l r) -> p l r",
                                                      r=CHUNK)
                    nc.vector.tensor_tensor(
                        out=r_out[:],
                        in0=r_in[:, :, 0:CHUNK],
                        in1=r_in[:, :, CHUNK:2 * CHUNK][:, :, ::-1],
                        op=Alu.min)
                    cur, other = other, cur
                    d = CHUNK // 2
                    while d >= 1:
                        dist_substage(half, d)
                        d //= 2
                    width = half

                nc.sync.dma_start(out=keys_d[t], in_=cur[:, :K].bitcast(u16))

    nc.compile()
    return nc


def _get_program():
    if "nc" not in _cached:
        _cached["nc"] = _build_program()
    return _cached["nc"]


def _make_in_maps(pos):
    pjb = np.ascontiguousarray(pos.T)  # [3, N]
    in_maps = []
    for cr in range(NCORES):
        rows0 = cr * ROWS_PER_CORE
        pit = pos[rows0: rows0 + ROWS_PER_CORE].reshape(NTILES, 128, 3)
        ig = (OFF + rows0 + np.arange(ROWS_PER_CORE, dtype=np.float32)
              ).reshape(NTILES, 128, 1).astype(np.float32)
        fl = (pit > 0.5).astype(np.float32)
        in_maps.append({
            "pjb": pjb,
            "pit": np.ascontiguousarray(pit),
            "ig": np.ascontiguousarray(ig),
            "fl": np.ascontiguousarray(fl),
        })
    return in_maps


def kernel(positions, cell, max_neighbours):
    from concourse.bass_utils import run_bass_kernel_spmd

    pos = np.asarray(positions, dtype=np.float32)
    assert pos.shape == (N, 3)
    k = int(max_neighbours)
    assert k == K, f"kernel hardcodes max_neighbours=256, got {k}"

    nc = _get_program()
    in_maps = _make_in_maps(pos)
    for attempt in range(3):  # transient axon/PJRT hiccups: retry
        try:
            res = run_bass_kernel_spmd(nc, in_maps,
                                       core_ids=list(range(NCORES)))
            break
        except Exception:
            if attempt == 2:
                raise
            import time
            time.sleep(2.0)

    keys = np.concatenate(
        [r["keys"].reshape(ROWS_PER_CORE, K) for r in res.results], axis=0)
    counts = np.concatenate(
        [r["cnt"].reshape(ROWS_PER_CORE) for r in res.results], axis=0)

    raw = keys.astype(np.int64)
    valid = raw != SENT
    key = raw - OFF
    j = key & (N - 1)
    cp = key >> 11
    bx = cp & 1
    by = (cp >> 1) & 1
    bz = cp >> 2
    f = pos > 0.5  # same rule as device flip2
    g = np.empty((N, K, 3), np.int64)
    for a, bbit in enumerate((bx, by, bz)):
        fa = f[:, a][:, None]
        g[:, :, a] = np.where(fa, np.where(bbit > 0, 1, 0),
                              np.where(bbit > 0, 0, -1))
    neighbours = np.where(valid, j, -1).astype(np.int32)
    cells = np.where(valid[..., None], g, 1).astype(np.int32)
    actual_max = np.int32(counts.max())
    return neighbours, cells, actual_max


# revision 12
# speedup vs baseline: 3.1898x; 1.0213x over previous
"""Periodic-boundary fixed-capacity neighbour list on 8 trn2 NeuronCores.

Device algorithm (per core, 256 rows as 2 partition-tiles):
  For unit cell + cutoff 0.3, a pair (i, j) is within cutoff for at most ONE
  of the 27 periodic images, and per axis at most TWO image shifts are ever
  feasible for a given centre row ({0,+1} if p_i>0.5 else {-1,0}).  So the
  N x 27N reference mask collapses to N x N with a 3-bit reduced cell code:
     v_a = p_j,a - p_i,a          (ACT FMA; compare vs +-0.5 immediates)
     g_a = [v<-0.5] - [v>0.5]     (threshold-epsilon differences are no-hit
                                   safe: |w| would be ~0.5 >> 0.3)
     u_a = g_a + p_j,a ;  w_a = u_a - p_i,a    (fp32, reference-exact order)
     hit = ((wx^2+wy^2)+wz^2) <= 0.09f  and  j != i
     b_a = [u_a - flip2_a >= 0]   (exact: Sterbenz),  flip2_a = [p_i,a>0.5]
     key = (bz*4+by*2+bx)*2048 + j + 8192  in [8192, 24576) ; miss -> 32512
  Ascending-key order == the reference argwhere packing order.  Keys are
  cast to uint16 and REINTERPRETED as bf16 (positive-float bit patterns
  sort like integers), so the bitonic top-256 (sort eight 256-chunks, then
  reversed-read prune-merges 8->4->2->1) runs mostly in the DVE 2x 16-bit
  mode.  Phase-1 is spread over ACT (FMA/Square, bitwise-IEEE verified),
  GPSIMD (tensor-tensor add/sub/mult only) and DVE immediate-scalar
  compares, so the DVE mostly runs the sort network.
Host: shard/replicate inputs, decode keys -> neighbours/cell_indices
(b-bit + flip rule -> shift vector), max of per-row hit counts ->
actual_max.  jnp.take(idx=-1) wraps: invalid cell slots = shifts[26] =
(1,1,1).
"""
import sys

if '/opt/trn_rl_repo' not in sys.path:
    sys.path.insert(0, '/opt/trn_rl_repo')

import numpy as np

N = 2048
K = 256
CHUNK = 256
NCORES = 8
ROWS_PER_CORE = N // NCORES  # 256
NTILES = ROWS_PER_CORE // 128  # 2
OFF = 8192
SENT = 32512  # 0x7F00 as uint16; huge finite positive as bf16
THR = 0.3 * 0.3  # fp32-converts to 0.090000004 like the jax reference

_cached = {}


def _build_program():
    import concourse.bacc as bacc
    import concourse.mybir as mybir
    from concourse.tile import TileContext

    f32 = mybir.dt.float32
    i32 = mybir.dt.int32
    u16 = mybir.dt.uint16
    bf16 = mybir.dt.bfloat16
    Alu = mybir.AluOpType
    Act = mybir.ActivationFunctionType
    Ax = mybir.AxisListType

    nc = bacc.Bacc("TRN2", target_bir_lowering=False)

    pjb_d = nc.dram_tensor("pjb", [3, N], f32, kind="ExternalInput")
    pit_d = nc.dram_tensor("pit", [NTILES, 128, 3], f32, kind="ExternalInput")
    ig_d = nc.dram_tensor("ig", [NTILES, 128, 1], f32, kind="ExternalInput")
    fl_d = nc.dram_tensor("fl", [NTILES, 128, 3], f32, kind="ExternalInput")
    keys_d = nc.dram_tensor("keys", [NTILES, 128, K], u16, kind="ExternalOutput")
    cnt_d = nc.dram_tensor("cnt", [NTILES, 128, 1], f32, kind="ExternalOutput")

    with TileContext(nc) as tc:
        with tc.tile_pool(name="main", bufs=1) as pool:
            big = [128, N]
            pj = [pool.tile(big, f32, name=f"pj{a}", tag=f"pj{a}") for a in range(3)]
            iota_i = pool.tile(big, i32, tag="iotai")
            iota_f = pool.tile(big, f32, tag="iotaf")

            for a in range(3):
                nc.sync.dma_start(
                    out=pj[a][:],
                    in_=pjb_d[a:a + 1, :].partition_broadcast(128).squeeze(1),
                )
            nc.gpsimd.iota(iota_i[:], pattern=[[1, N]], base=OFF,
                           channel_multiplier=0)
            nc.vector.tensor_copy(out=iota_f[:], in_=iota_i[:])

            sa = [pool.tile(big, f32, name=f"sa{a}", tag=f"sa{a}")
                  for a in range(3)]
            AB = {}
            P1 = {}

            def head(t):
                pit_s = pool.tile([128, 3], f32, tag=f"pit{t}")
                nig_s = pool.tile([128, 1], f32, tag=f"nig{t}")
                nfl_s = pool.tile([128, 3], f32, tag=f"nfl{t}")
                npi_s = pool.tile([128, 3], f32, tag=f"npi{t}")
                cnt_s = pool.tile([128, 1], f32, tag=f"cnt{t}")
                nc.sync.dma_start(out=pit_s[:], in_=pit_d[t])
                ig_s = pool.tile([128, 1], f32, tag=f"ig{t}")
                fl_s = pool.tile([128, 3], f32, tag=f"fl{t}")
                nc.sync.dma_start(out=ig_s[:], in_=ig_d[t])
                nc.sync.dma_start(out=fl_s[:], in_=fl_d[t])
                nc.vector.tensor_scalar(out=npi_s[:], in0=pit_s[:], scalar1=-1.0,
                                        scalar2=None, op0=Alu.mult)
                nc.vector.tensor_scalar(out=nig_s[:], in0=ig_s[:], scalar1=-1.0,
                                        scalar2=None, op0=Alu.mult)
                nc.vector.tensor_scalar(out=nfl_s[:], in0=fl_s[:], scalar1=-1.0,
                                        scalar2=None, op0=Alu.mult)
                P1[t] = (npi_s, nig_s, nfl_s, cnt_s)
                # per-axis: v = pj - pi [ACT]; compares [DVE]; g [GPSIMD]
                for a in range(3):
                    va = pool.tile(big, f32, name=f"va{a}", tag="va")
                    cma = pool.tile(big, f32, name=f"cma{a}", tag="cma")
                    cmb = pool.tile(big, f32, name=f"cmb{a}", tag="cmb")
                    nc.scalar.activation(va[:], pj[a][:], Act.Identity,
                                         bias=npi_s[:, a:a + 1], scale=1.0)
                    nc.vector.tensor_scalar(out=cma[:], in0=va[:],
                                            scalar1=-0.5, scalar2=None,
                                            op0=Alu.is_lt)
                    nc.vector.tensor_scalar(out=cmb[:], in0=va[:],
                                            scalar1=0.5, scalar2=None,
                                            op0=Alu.is_gt)
                    nc.gpsimd.tensor_tensor(out=sa[a][:], in0=cma[:],
                                            in1=cmb[:], op=Alu.subtract)

            def tail(t):
                npi_s, nig_s, nfl_s, cnt_s = P1[t]
                u = [pool.tile(big, f32, name=f"u{a}", tag=f"u{a}") for a in range(3)]
                bt = [pool.tile(big, f32, name=f"bt{a}", tag=f"bt{a}") for a in range(3)]
                t1 = pool.tile(big, f32, tag="t1")
                t2 = pool.tile(big, f32, tag="t2")
                d2 = pool.tile(big, f32, tag="d2")
                hitm = pool.tile(big, f32, tag="hitm")
                noteq = pool.tile(big, f32, tag="iotai")  # reuse iota_i slot

                for a in range(3):
                    uba = pool.tile(big, f32, name=f"uba{a}", tag="uba")
                    # u = g + pj   [GPSIMD]
                    nc.gpsimd.tensor_tensor(out=u[a][:], in0=sa[a][:],
                                            in1=pj[a][:], op=Alu.add)
                    # order bit: (u - flip2 >= 0) * 2^(11+a)
                    nc.scalar.activation(uba[:], u[a][:], Act.Identity,
                                         bias=nfl_s[:, a:a + 1], scale=1.0)
                    nc.vector.tensor_scalar(out=bt[a][:], in0=uba[:],
                                            scalar1=0.0,
                                            scalar2=float(2048 << a),
                                            op0=Alu.is_ge, op1=Alu.mult)
                    # w = u - pi ; sq = w^2  [ACT, bit-exact, in-place]
                    nc.scalar.activation(u[a][:], u[a][:], Act.Identity,
                                         bias=npi_s[:, a:a + 1], scale=1.0)
                    nc.scalar.activation(u[a][:], u[a][:], Act.Square)

                # d2 = (sq0 + sq1) + sq2   [DVE]
                nc.vector.tensor_tensor(out=t2[:], in0=u[0][:], in1=u[1][:],
                                        op=Alu.add)
                nc.vector.tensor_tensor(out=d2[:], in0=t2[:], in1=u[2][:],
                                        op=Alu.add)
                nc.vector.tensor_scalar(out=t2[:], in0=d2[:], scalar1=THR,
                                        scalar2=None, op0=Alu.is_le)
                vn = pool.tile(big, f32, name=f"vn{t}", tag="va")
                nc.scalar.activation(vn[:], iota_f[:], Act.Identity,
                                     bias=nig_s[:], scale=1.0)
                nc.vector.tensor_scalar(out=noteq[:], in0=vn[:], scalar1=0.0,
                                        scalar2=None, op0=Alu.not_equal)
                nc.vector.tensor_tensor(out=hitm[:], in0=t2[:], in1=noteq[:],
                                        op=Alu.mult)
                nc.vector.tensor_reduce(out=cnt_s[:], in_=hitm[:], axis=Ax.X,
                                        op=Alu.add)
                nc.sync.dma_start(out=cnt_d[t], in_=cnt_s[:])

                nc.vector.tensor_tensor(out=t1[:], in0=bt[0][:], in1=bt[1][:],
                                        op=Alu.add)
                nc.vector.tensor_tensor(out=t2[:], in0=t1[:], in1=bt[2][:],
                                        op=Alu.add)
                nc.vector.tensor_tensor(out=t1[:], in0=t2[:], in1=iota_f[:],
                                        op=Alu.add)
                nc.vector.scalar_tensor_tensor(out=t2[:], in0=t1[:],
                                               scalar=float(SENT),
                                               in1=hitm[:], op0=Alu.subtract,
                                               op1=Alu.mult)
                A = pool.tile(big, bf16, name=f"A{t}", tag=f"A{t}")
                B = pool.tile(big, bf16, name=f"B{t}", tag=f"B{t}")
                AB[t] = (A, B)
                nc.scalar.activation(A[:].bitcast(u16), t2[:], Act.Copy,
                                     bias=float(SENT))

            def sort(t):
                cur, other = AB[t]

                def substage(lo_in, hi_in, lo_out, hi_out):
                    nonlocal cur, other
                    nc.vector.tensor_tensor(out=lo_out, in0=lo_in, in1=hi_in,
                                            op=Alu.min)
                    nc.vector.tensor_tensor(out=hi_out, in0=lo_in, in1=hi_in,
                                            op=Alu.max)
                    cur, other = other, cur

                def dist_substage(width, d):
                    r_in = cur[:, :width].rearrange("p (b r) -> p b r", r=2 * d)
                    r_out = other[:, :width].rearrange("p (b r) -> p b r", r=2 * d)
                    substage(r_in[:, :, 0:d], r_in[:, :, d:2 * d],
                             r_out[:, :, 0:d], r_out[:, :, d:2 * d])

                for mexp in range(8):
                    m = 1 << mexp
                    r_in = cur[:].rearrange("p (b r) -> p b r", r=2 * m)
                    r_out = other[:].rearrange("p (b r) -> p b r", r=2 * m)
                    substage(r_in[:, :, 0:m], r_in[:, :, m:2 * m][:, :, ::-1],
                             r_out[:, :, 0:m], r_out[:, :, m:2 * m][:, :, ::-1])
                    d = m // 2
                    while d >= 1:
                        dist_substage(N, d)
                        d //= 2

                width = N
                while width > CHUNK:
                    half = width // 2
                    r_in = cur[:, :width].rearrange("p (l r) -> p l r",
                                                    r=2 * CHUNK)
                    r_out = other[:, :half].rearrange("p (l r) -> p l r",
                                                      r=CHUNK)
                    nc.vector.tensor_tensor(
                        out=r_out[:],
                        in0=r_in[:, :, 0:CHUNK],
                        in1=r_in[:, :, CHUNK:2 * CHUNK][:, :, ::-1],
                        op=Alu.min)
                    cur, other = other, cur
                    d = CHUNK // 2
                    while d >= 1:
                        dist_substage(half, d)
                        d //= 2
                    width = half

                nc.sync.dma_start(out=keys_d[t], in_=cur[:, :K].bitcast(u16))

            head(0)
            tail(0)
            head(1)
            sort(0)
            tail(1)
            sort(1)

    nc.compile()
    return nc


def _get_program():
    if "nc" not in _cached:
        _cached["nc"] = _build_program()
    return _cached["nc"]


def _make_in_maps(pos):
    pjb = np.ascontiguousarray(pos.T)  # [3, N]
    in_maps = []
    for cr in range(NCORES):
        rows0 = cr * ROWS_PER_CORE
        pit = pos[rows0: rows0 + ROWS_PER_CORE].reshape(NTILES, 128, 3)
        ig = (OFF + rows0 + np.arange(ROWS_PER_CORE, dtype=np.float32)
              ).reshape(NTILES, 128, 1).astype(np.float32)
        fl = (pit > 0.5).astype(np.float32)
        in_maps.append({
            "pjb": pjb,
            "pit": np.ascontiguousarray(pit),
            "ig": np.ascontiguousarray(ig),
            "fl": np.ascontiguousarray(fl),
        })
    return in_maps


def kernel(positions, cell, max_neighbours):
    from concourse.bass_utils import run_bass_kernel_spmd

    pos = np.asarray(positions, dtype=np.float32)
    assert pos.shape == (N, 3)
    k = int(max_neighbours)
    assert k == K, f"kernel hardcodes max_neighbours=256, got {k}"

    nc = _get_program()
    in_maps = _make_in_maps(pos)
    for attempt in range(3):  # transient axon/PJRT hiccups: retry
        try:
            res = run_bass_kernel_spmd(nc, in_maps,
                                       core_ids=list(range(NCORES)))
            break
        except Exception:
            if attempt == 2:
                raise
            import time
            time.sleep(2.0)

    keys = np.concatenate(
        [r["keys"].reshape(ROWS_PER_CORE, K) for r in res.results], axis=0)
    counts = np.concatenate(
        [r["cnt"].reshape(ROWS_PER_CORE) for r in res.results], axis=0)

    raw = keys.astype(np.int64)
    valid = raw != SENT
    key = raw - OFF
    j = key & (N - 1)
    cp = key >> 11
    bx = cp & 1
    by = (cp >> 1) & 1
    bz = cp >> 2
    f = pos > 0.5  # same rule as device flip2
    g = np.empty((N, K, 3), np.int64)
    for a, bbit in enumerate((bx, by, bz)):
        fa = f[:, a][:, None]
        g[:, :, a] = np.where(fa, np.where(bbit > 0, 1, 0),
                              np.where(bbit > 0, 0, -1))
    neighbours = np.where(valid, j, -1).astype(np.int32)
    cells = np.where(valid[..., None], g, 1).astype(np.int32)
    actual_max = np.int32(counts.max())
    return neighbours, cells, actual_max


# revision 13
# speedup vs baseline: 3.2252x; 1.0111x over previous
"""Periodic-boundary fixed-capacity neighbour list on 8 trn2 NeuronCores.

Device algorithm (per core, 256 rows as 2 partition-tiles):
  For unit cell + cutoff 0.3, a pair (i, j) is within cutoff for at most ONE
  of the 27 periodic images, and per axis at most TWO image shifts are ever
  feasible for a given centre row ({0,+1} if p_i>0.5 else {-1,0}).  So the
  N x 27N reference mask collapses to N x N with a 3-bit reduced cell code:
     v_a = p_j,a - p_i,a          (ACT FMA; compare vs +-0.5 immediates)
     g_a = [v<-0.5] - [v>0.5]     (threshold-epsilon differences are no-hit
                                   safe: |w| would be ~0.5 >> 0.3)
     u_a = g_a + p_j,a ;  w_a = u_a - p_i,a    (fp32, reference-exact order)
     hit = ((wx^2+wy^2)+wz^2) <= 0.09f  and  j != i
     b_a = [u_a - flip2_a >= 0]   (exact: Sterbenz),  flip2_a = [p_i,a>0.5]
     key = (bz*4+by*2+bx)*2048 + j + 8192  in [8192, 24576) ; miss -> 32512
  Ascending-key order == the reference argwhere packing order.  Keys are
  cast to uint16 and REINTERPRETED as bf16 (positive-float bit patterns
  sort like integers), so the bitonic top-256 (sort eight 256-chunks, then
  reversed-read prune-merges 8->4->2->1) runs mostly in the DVE 2x 16-bit
  mode.  Phase-1 is spread over ACT (FMA/Square, bitwise-IEEE verified),
  GPSIMD (tensor-tensor add/sub/mult only) and DVE immediate-scalar
  compares, so the DVE mostly runs the sort network.
Host: shard/replicate inputs, decode keys -> neighbours/cell_indices
(b-bit + flip rule -> shift vector), max of per-row hit counts ->
actual_max.  jnp.take(idx=-1) wraps: invalid cell slots = shifts[26] =
(1,1,1).
"""
import sys

if '/opt/trn_rl_repo' not in sys.path:
    sys.path.insert(0, '/opt/trn_rl_repo')

import numpy as np

N = 2048
K = 256
CHUNK = 256
NCORES = 8
ROWS_PER_CORE = N // NCORES  # 256
NTILES = ROWS_PER_CORE // 128  # 2
OFF = 8192
SENT = 32512  # 0x7F00 as uint16; huge finite positive as bf16
THR = 0.3 * 0.3  # fp32-converts to 0.090000004 like the jax reference

_cached = {}


def _build_program():
    import concourse.bacc as bacc
    import concourse.mybir as mybir
    from concourse.tile import TileContext

    f32 = mybir.dt.float32
    i32 = mybir.dt.int32
    u16 = mybir.dt.uint16
    bf16 = mybir.dt.bfloat16
    Alu = mybir.AluOpType
    Act = mybir.ActivationFunctionType
    Ax = mybir.AxisListType

    nc = bacc.Bacc("TRN2", target_bir_lowering=False)

    pjb_d = nc.dram_tensor("pjb", [3, N], f32, kind="ExternalInput")
    pit_d = nc.dram_tensor("pit", [NTILES, 128, 3], f32, kind="ExternalInput")
    ig_d = nc.dram_tensor("ig", [NTILES, 128, 1], f32, kind="ExternalInput")
    fl_d = nc.dram_tensor("fl", [NTILES, 128, 3], f32, kind="ExternalInput")
    keys_d = nc.dram_tensor("keys", [NTILES, 128, K], u16, kind="ExternalOutput")
    cnt_d = nc.dram_tensor("cnt", [NTILES, 128, 1], f32, kind="ExternalOutput")

    with TileContext(nc) as tc:
        with tc.tile_pool(name="main", bufs=1) as pool:
            big = [128, N]
            pj = [pool.tile(big, f32, name=f"pj{a}", tag=f"pj{a}") for a in range(3)]
            iota_i = pool.tile(big, i32, tag="iotai")
            iota_f = pool.tile(big, f32, tag="iotaf")

            for a in range(3):
                nc.sync.dma_start(
                    out=pj[a][:],
                    in_=pjb_d[a:a + 1, :].partition_broadcast(128).squeeze(1),
                )
            nc.gpsimd.iota(iota_i[:], pattern=[[1, N]], base=OFF,
                           channel_multiplier=0)
            nc.vector.tensor_copy(out=iota_f[:], in_=iota_i[:])

            sa = [pool.tile(big, f32, name=f"sa{a}", tag=f"sa{a}")
                  for a in range(3)]
            AB = {}
            P1 = {}

            def head(t):
                pit_s = pool.tile([128, 3], f32, tag=f"pit{t}")
                nig_s = pool.tile([128, 1], f32, tag=f"nig{t}")
                nfl_s = pool.tile([128, 3], f32, tag=f"nfl{t}")
                npi_s = pool.tile([128, 3], f32, tag=f"npi{t}")
                cnt_s = pool.tile([128, 1], f32, tag=f"cnt{t}")
                nc.sync.dma_start(out=pit_s[:], in_=pit_d[t])
                ig_s = pool.tile([128, 1], f32, tag=f"ig{t}")
                fl_s = pool.tile([128, 3], f32, tag=f"fl{t}")
                nc.sync.dma_start(out=ig_s[:], in_=ig_d[t])
                nc.sync.dma_start(out=fl_s[:], in_=fl_d[t])
                nc.vector.tensor_scalar(out=npi_s[:], in0=pit_s[:], scalar1=-1.0,
                                        scalar2=None, op0=Alu.mult)
                nc.vector.tensor_scalar(out=nig_s[:], in0=ig_s[:], scalar1=-1.0,
                                        scalar2=None, op0=Alu.mult)
                nc.vector.tensor_scalar(out=nfl_s[:], in0=fl_s[:], scalar1=-1.0,
                                        scalar2=None, op0=Alu.mult)
                P1[t] = (npi_s, nig_s, nfl_s, cnt_s)
                # per-axis: v = pj - pi [ACT]; compares [DVE]; g [GPSIMD]
                for a in range(3):
                    va = pool.tile(big, f32, name=f"va{a}", tag="va")
                    cma = pool.tile(big, f32, name=f"cma{a}", tag="cma")
                    cmb = pool.tile(big, f32, name=f"cmb{a}", tag="cmb")
                    nc.scalar.activation(va[:], pj[a][:], Act.Identity,
                                         bias=npi_s[:, a:a + 1], scale=1.0)
                    nc.vector.tensor_scalar(out=cma[:], in0=va[:],
                                            scalar1=-0.5, scalar2=None,
                                            op0=Alu.is_lt)
                    nc.vector.tensor_scalar(out=cmb[:], in0=va[:],
                                            scalar1=0.5, scalar2=None,
                                            op0=Alu.is_gt)
                    nc.gpsimd.tensor_tensor(out=sa[a][:], in0=cma[:],
                                            in1=cmb[:], op=Alu.subtract)

            def tail(t):
                npi_s, nig_s, nfl_s, cnt_s = P1[t]
                u = [pool.tile(big, f32, name=f"u{a}", tag=f"u{a}") for a in range(3)]
                bt = [pool.tile(big, f32, name=f"bt{a}", tag=f"bt{a}") for a in range(3)]
                t1 = pool.tile(big, f32, tag="t1")
                t2 = pool.tile(big, f32, tag="t2")
                d2 = pool.tile(big, f32, tag="d2")
                hitm = pool.tile(big, f32, tag="hitm")
                noteq = pool.tile(big, f32, tag="iotai")  # reuse iota_i slot

                for a in range(3):
                    uba = pool.tile(big, f32, name=f"uba{a}", tag="uba")
                    # u = g + pj   [GPSIMD]
                    nc.gpsimd.tensor_tensor(out=u[a][:], in0=sa[a][:],
                                            in1=pj[a][:], op=Alu.add)
                    # order bit: (u - flip2 >= 0) * 2^(11+a)
                    nc.scalar.activation(uba[:], u[a][:], Act.Identity,
                                         bias=nfl_s[:, a:a + 1], scale=1.0)
                    nc.vector.tensor_scalar(out=bt[a][:], in0=uba[:],
                                            scalar1=0.0,
                                            scalar2=float(2048 << a),
                                            op0=Alu.is_ge, op1=Alu.mult)
                    # w = u - pi ; sq = w^2  [ACT, bit-exact, in-place]
                    nc.scalar.activation(u[a][:], u[a][:], Act.Identity,
                                         bias=npi_s[:, a:a + 1], scale=1.0)
                    nc.scalar.activation(u[a][:], u[a][:], Act.Square)

                # d2 = (sq0 + sq1) + sq2   [DVE]
                nc.vector.tensor_tensor(out=t2[:], in0=u[0][:], in1=u[1][:],
                                        op=Alu.add)
                nc.vector.tensor_tensor(out=d2[:], in0=t2[:], in1=u[2][:],
                                        op=Alu.add)
                nc.vector.tensor_scalar(out=t2[:], in0=d2[:], scalar1=THR,
                                        scalar2=None, op0=Alu.is_le)
                vn = pool.tile(big, f32, name=f"vn{t}", tag="va")
                nc.scalar.activation(vn[:], iota_f[:], Act.Identity,
                                     bias=nig_s[:], scale=1.0)
                nc.vector.tensor_scalar(out=noteq[:], in0=vn[:], scalar1=0.0,
                                        scalar2=None, op0=Alu.not_equal)
                nc.vector.tensor_tensor(out=hitm[:], in0=t2[:], in1=noteq[:],
                                        op=Alu.mult)
                nc.vector.tensor_reduce(out=cnt_s[:], in_=hitm[:], axis=Ax.X,
                                        op=Alu.add)
                nc.sync.dma_start(out=cnt_d[t], in_=cnt_s[:])

                nc.vector.tensor_tensor(out=t1[:], in0=bt[0][:], in1=bt[1][:],
                                        op=Alu.add)
                nc.vector.tensor_tensor(out=t2[:], in0=t1[:], in1=bt[2][:],
                                        op=Alu.add)
                nc.vector.tensor_tensor(out=t1[:], in0=t2[:], in1=iota_f[:],
                                        op=Alu.add)
                nc.vector.scalar_tensor_tensor(out=t2[:], in0=t1[:],
                                               scalar=float(SENT),
                                               in1=hitm[:], op0=Alu.subtract,
                                               op1=Alu.mult)
                A = pool.tile(big, bf16, name=f"A{t}", tag=f"A{t}")
                B = pool.tile(big, bf16, name=f"B{t}", tag=f"B{t}")
                AB[t] = (A, B)
                nc.scalar.activation(A[:].bitcast(u16), t2[:], Act.Copy,
                                     bias=float(SENT))

            def sort_gen(t):
                cur, other = AB[t]
                state = [cur, other]

                def substage(lo_in, hi_in, lo_out, hi_out):
                    nc.vector.tensor_tensor(out=lo_out, in0=lo_in, in1=hi_in,
                                            op=Alu.min)
                    nc.vector.tensor_tensor(out=hi_out, in0=lo_in, in1=hi_in,
                                            op=Alu.max)
                    state[0], state[1] = state[1], state[0]

                def dist_substage(width, d):
                    cur, other = state
                    r_in = cur[:, :width].rearrange("p (b r) -> p b r", r=2 * d)
                    r_out = other[:, :width].rearrange("p (b r) -> p b r", r=2 * d)
                    substage(r_in[:, :, 0:d], r_in[:, :, d:2 * d],
                             r_out[:, :, 0:d], r_out[:, :, d:2 * d])

                for mexp in range(8):
                    m = 1 << mexp
                    cur, other = state
                    r_in = cur[:].rearrange("p (b r) -> p b r", r=2 * m)
                    r_out = other[:].rearrange("p (b r) -> p b r", r=2 * m)
                    substage(r_in[:, :, 0:m], r_in[:, :, m:2 * m][:, :, ::-1],
                             r_out[:, :, 0:m], r_out[:, :, m:2 * m][:, :, ::-1])
                    yield
                    d = m // 2
                    while d >= 1:
                        dist_substage(N, d)
                        yield
                        d //= 2

                width = N
                while width > CHUNK:
                    half = width // 2
                    cur, other = state
                    r_in = cur[:, :width].rearrange("p (l r) -> p l r",
                                                    r=2 * CHUNK)
                    r_out = other[:, :half].rearrange("p (l r) -> p l r",
                                                      r=CHUNK)
                    nc.vector.tensor_tensor(
                        out=r_out[:],
                        in0=r_in[:, :, 0:CHUNK],
                        in1=r_in[:, :, CHUNK:2 * CHUNK][:, :, ::-1],
                        op=Alu.min)
                    state[0], state[1] = state[1], state[0]
                    yield
                    d = CHUNK // 2
                    while d >= 1:
                        dist_substage(half, d)
                        yield
                        d //= 2
                    width = half

                nc.sync.dma_start(out=keys_d[t],
                                  in_=state[0][:, :K].bitcast(u16))
                yield

            head(0)
            tail(0)
            head(1)
            g0 = sort_gen(0)
            # run a prefix of sort(0) while ACT/GPSIMD chew tile 1's chain
            for _ in range(16):
                next(g0)
            tail(1)
            g1 = sort_gen(1)
            done0 = done1 = False
            while not (done0 and done1):
                if not done0:
                    try:
                        next(g0)
                    except StopIteration:
                        done0 = True
                if not done1:
                    try:
                        next(g1)
                    except StopIteration:
                        done1 = True

    nc.compile()
    return nc


def _get_program():
    if "nc" not in _cached:
        _cached["nc"] = _build_program()
    return _cached["nc"]


def _make_in_maps(pos):
    pjb = np.ascontiguousarray(pos.T)  # [3, N]
    in_maps = []
    for cr in range(NCORES):
        rows0 = cr * ROWS_PER_CORE
        pit = pos[rows0: rows0 + ROWS_PER_CORE].reshape(NTILES, 128, 3)
        ig = (OFF + rows0 + np.arange(ROWS_PER_CORE, dtype=np.float32)
              ).reshape(NTILES, 128, 1).astype(np.float32)
        fl = (pit > 0.5).astype(np.float32)
        in_maps.append({
            "pjb": pjb,
            "pit": np.ascontiguousarray(pit),
            "ig": np.ascontiguousarray(ig),
            "fl": np.ascontiguousarray(fl),
        })
    return in_maps


def kernel(positions, cell, max_neighbours):
    from concourse.bass_utils import run_bass_kernel_spmd

    pos = np.asarray(positions, dtype=np.float32)
    assert pos.shape == (N, 3)
    k = int(max_neighbours)
    assert k == K, f"kernel hardcodes max_neighbours=256, got {k}"

    nc = _get_program()
    in_maps = _make_in_maps(pos)
    for attempt in range(3):  # transient axon/PJRT hiccups: retry
        try:
            res = run_bass_kernel_spmd(nc, in_maps,
                                       core_ids=list(range(NCORES)))
            break
        except Exception:
            if attempt == 2:
                raise
            import time
            time.sleep(2.0)

    keys = np.concatenate(
        [r["keys"].reshape(ROWS_PER_CORE, K) for r in res.results], axis=0)
    counts = np.concatenate(
        [r["cnt"].reshape(ROWS_PER_CORE) for r in res.results], axis=0)

    raw = keys.astype(np.int64)
    valid = raw != SENT
    key = raw - OFF
    j = key & (N - 1)
    cp = key >> 11
    bx = cp & 1
    by = (cp >> 1) & 1
    bz = cp >> 2
    f = pos > 0.5  # same rule as device flip2
    g = np.empty((N, K, 3), np.int64)
    for a, bbit in enumerate((bx, by, bz)):
        fa = f[:, a][:, None]
        g[:, :, a] = np.where(fa, np.where(bbit > 0, 1, 0),
                              np.where(bbit > 0, 0, -1))
    neighbours = np.where(valid, j, -1).astype(np.int32)
    cells = np.where(valid[..., None], g, 1).astype(np.int32)
    actual_max = np.int32(counts.max())
    return neighbours, cells, actual_max
